# revision 4
# baseline (speedup 1.0000x reference)
"""Multi-head attention (B=4, S=2048, D=1024, H=16) on 8 Trainium2 cores. v2

Sharding: core c -> (batch b=c//2, query-half hq=c%2). Each core computes
K/V projections for its batch's full sequence (no collectives needed) and
attention + output projection for its 1024 query rows.

v2 changes over the 432us baseline:
  * scores: two concurrent K=64 matmuls via PE row-tiling (tile_position
    (0,0)/(64,0) auto-derived from base partitions) instead of zero-padded
    K=128 contractions -- halves the scores PE time.  Iteration unit is
    (qc, head-pair ht, k-tile kt); the [128,1024] scores psum holds
    [head-even | head-odd] halves so the exp stays one [128,1024] ACT op.
  * q kept packed [128, SQ] per head-pair (qT2), no zero-padding memsets.
  * ctx normalization transposes moved from PE (identity matmuls through
    PSUM) to DVE 32x32 StreamTranspose ops -- frees ~35us of PE and one
    PSUM bank, letting scores/ctx/out-proj use all 8 banks.
  * x DMA'd chunk-major (s-chunks of 512 across all d-tiles) so the first
    K-projection group starts after 1MB instead of 4MB.
  * qc1 out-proj split: dt 0..6 partial-accumulated into SBUF during the
    last head's iterations, only the dt=7 matmul + DVE add in the tail.

Device dataflow (activations kept transposed, [feature, seq]):
  kT[e,s]   = WkT.T-contract  (lhsT=WkT[d,e] tiles, rhs=xT[d,s])
  qT2[ht]   = packed per-pair q [128, SQ]: head 2ht at partitions 0:64,
              head 2ht+1 at partitions 64:128.
  v[s,e]    = lhsT=xT[d,s] tiles, rhs=WvT[d,e]  (+bias via DVE add of a
              partition-broadcast bv); col 64 of each head = ones
              (softmax denominator column).
  per (qc, ht, kt):
    scoresT[k,q] for both heads of ht concurrently (row groups 0/64)
    expT = ScalarE Exp -> bf16 sbuf [128, 1024]
    ctx (flipped): ctq_h[q, hd+1] += expT-slice (stationary) @ vv[kt][:,h,:]
  norm: reciprocal_approx_fast [128,4] + per-qi tensor_scalar_mul -> ctqn
        bf16, then 8 DVE StreamTranspose ops scatter [q,d] -> ctxn[d,q].
  outT[e,q] = WoT.T-contract ctxn  (bias bo added host-side)
Host: out[b, hq*1024:(hq+1)*1024, :] = outT.T + bo
"""

import numpy as np
import ml_dtypes

import concourse.bacc as bacc
import concourse.tile as tile
from concourse import mybir
from concourse.bass_utils import run_bass_kernel_spmd
from concourse.masks import make_identity

B, S, D = 4, 2048, 1024
H, HD = 16, 64
SQ = 1024          # query rows per core
NDT = D // 128     # 8 d-tiles
NET = D // 128     # 8 e-tiles
NKT = S // 128     # 16 k-tiles
NST = S // 128     # 16 s-tiles
NQC = SQ // 512    # 2 q-chunks per core
NHT = H // 2       # 8 head-pairs
BF16 = mybir.dt.bfloat16
F32 = mybir.dt.float32
SCALE = 1.0 / 8.0  # 1/sqrt(HD)

_NC_CACHE = None


def build_nc():
    nc = bacc.Bacc(None, target_bir_lowering=False, debug=True)

    xT_d = nc.declare_dram_parameter("xT", [D, S], BF16, isOutput=False)
    WqT_d = nc.declare_dram_parameter("WqT", [D, D], BF16, isOutput=False)
    WkT_d = nc.declare_dram_parameter("WkT", [D, D], BF16, isOutput=False)
    WvT_d = nc.declare_dram_parameter("WvT", [D, D], BF16, isOutput=False)
    WoT_d = nc.declare_dram_parameter("WoT", [D, D], BF16, isOutput=False)
    bqt_d = nc.declare_dram_parameter("bqt", [128, NET], F32, isOutput=False)
    bkt_d = nc.declare_dram_parameter("bkt", [128, NET], F32, isOutput=False)
    bvr_d = nc.declare_dram_parameter("bvr", [1, D], F32, isOutput=False)
    outT_d = nc.declare_dram_parameter("outT", [D, SQ], F32, isOutput=True)

    VC = 256           # v-projection chunk width (4 heads per chunk)
    NVC = D // VC      # 4 chunks

    with tile.TileContext(nc) as tc:
        with tc.tile_pool(name="resident", bufs=1) as res:
            # ---- resident SBUF tensors ----
            kT = [res.tile([128, S], BF16, name=f"kT{t}", tag=f"kT{t}")
                  for t in range(NET)]
            qT2 = [res.tile([128, SQ], BF16, name=f"qT2_{t}", tag=f"qT2_{t}")
                   for t in range(NHT)]
            vv = [res.tile([128, H, HD + 1], BF16, name=f"v{t}", tag=f"v{t}")
                  for t in range(NST)]
            ctxn = [[res.tile([128, 512], BF16, name=f"ctxn{qc}_{t}",
                              tag=f"ctxn{qc}_{t}") for t in range(NDT)]
                    for qc in range(NQC)]
            Wo_t = [res.tile([128, D], BF16, name=f"Wo{t}", tag=f"Wo{t}")
                    for t in range(NDT)]
            xT = [res.tile([128, S], BF16, name=f"xT{t}", tag=f"xT{t}")
                  for t in range(NDT)]
            osb_part = [res.tile([128, 512], BF16, name=f"osbp{t}",
                                 tag=f"osbp{t}") for t in range(NET)]
            bq_dma = res.tile([128, NET], F32, tag="bq_dma")
            bk_dma = res.tile([128, NET], F32, tag="bk_dma")
            bq_sb = res.tile([128, NET], F32, tag="bq_sb")
            bk_sb = res.tile([128, NET], F32, tag="bk_sb")
            bv_sb = res.tile([1, D], F32, tag="bv_sb")
            bv_bc = res.tile([128, D], F32, tag="bv_bc")
            ident = res.tile([128, 128], BF16, tag="ident")

            # identity first: it alone gates the PE warmup spins, which in
            # turn cover the several-us engine/DMA startup latency
            ones_c = res.tile([1, 128], F32, tag="ones_c")
            make_identity(nc, ident)
            nc.vector.memset(ones_c, 1.0)
            nc.sync.dma_start(out=bq_dma, in_=bqt_d[:, :])
            nc.sync.dma_start(out=bk_dma, in_=bkt_d[:, :])
            nc.sync.dma_start(out=bv_sb, in_=bvr_d[:, :])
            # TensorScalarPtr has a single sync-wait slot; route the biases
            # through DVE once so later readers rely on program order.
            nc.vector.tensor_copy(out=bq_sb, in_=bq_dma)
            nc.vector.tensor_copy(out=bk_sb, in_=bk_dma)
            for t in range(NST):
                # only the denominator column; cols 0:HD are overwritten
                nc.vector.memset(vv[t][:, :, HD:HD + 1], 1.0)

            with tc.tile_pool(name="p2", bufs=1) as p2:
                psum_src = {}

                def proj_ps():
                    return psum_src["pool"].tile(
                        [128, 512], F32, name="ps", tag=psum_src["tag"],
                        bufs=psum_src["bufs"])

                # ---------- projection emitters (also used as fillers) ----
                # weight SLICES are DMA-streamed per e-tile/chunk so that
                # three full weight sets never have to live in SBUF at once
                wk_cache = {}
                wq_cache = {}
                wv_cache = {}

                def w_slices(cache, key, W_d, c0, c1, tag, bufs):
                    if key not in cache:
                        ws = []
                        for dt in range(NDT):
                            wt = p2.tile([128, c1 - c0], BF16,
                                         name=f"{tag}{dt}", tag=tag,
                                         bufs=bufs)
                            nc.sync.dma_start(
                                out=wt, in_=W_d[dt * 128:(dt + 1) * 128,
                                                c0:c1])
                            ws.append(wt)
                        cache.clear()
                        cache[key] = ws
                    return cache[key]

                def emit_k_group(et, sc):
                    ws = w_slices(wk_cache, ("k", et), WkT_d,
                                  et * 128, (et + 1) * 128, "wks", 18)
                    ps = proj_ps()
                    for dt in range(NDT):
                        nc.tensor.matmul(
                            ps, ws[dt],
                            xT[dt][:, sc * 512: sc * 512 + 512],
                            start=(dt == 0), stop=(dt == NDT - 1))
                    nc.vector.tensor_scalar_add(
                        out=kT[et][:, sc * 512:(sc + 1) * 512],
                        in0=ps,
                        scalar1=bk_sb[:, et:et + 1])

                def emit_q_group(et, sc):
                    ws = w_slices(wq_cache, ("q", et), WqT_d,
                                  et * 128, (et + 1) * 128, "wqs", 18)
                    ps = proj_ps()
                    for dt in range(NDT):
                        nc.tensor.matmul(
                            ps, ws[dt],
                            xT[dt][:, sc * 512: sc * 512 + 512],
                            start=(dt == 0), stop=(dt == NDT - 1))
                    sl = slice(sc * 512, (sc + 1) * 512)
                    nc.vector.tensor_scalar_add(
                        out=qT2[et][0:64, sl],
                        in0=ps[0:64, :],
                        scalar1=bq_sb[0:64, et:et + 1])
                    nc.vector.tensor_scalar_add(
                        out=qT2[et][64:128, sl],
                        in0=ps[64:128, :],
                        scalar1=bq_sb[64:128, et:et + 1])

                def emit_v_group(st, c):
                    # v chunk c covers e-columns [c*VC, (c+1)*VC) = 4 heads
                    ws = w_slices(wv_cache, ("v", c), WvT_d,
                                  c * VC, (c + 1) * VC, "wvs", 18)
                    psw = proj_ps()
                    ps = psw[:, 0:VC]
                    for dt in range(NDT):
                        nc.tensor.matmul(
                            ps,
                            xT[dt][:, st * 128:(st + 1) * 128],
                            ws[dt],
                            start=(dt == 0), stop=(dt == NDT - 1))
                    nh = VC // HD
                    nc.vector.tensor_add(
                        out=vv[st][:, c * nh:(c + 1) * nh, 0:HD],
                        in0=ps.rearrange("p (h d) -> p h d", h=nh),
                        in1=bv_bc[:, c * VC:(c + 1) * VC].rearrange(
                            "p (h d) -> p h d", h=nh))

                # ---------- upfront: weights-for-k0/q0 and x chunk-major --
                # startup-critical DMA order: k0/q0 weight slices, then x
                # s-chunk-major so each k0 projection group only waits on
                # its chunk's 8 tiles (subtile deps)
                w_slices(wk_cache, ("k", 0), WkT_d, 0, 128, "wks", 18)
                w_slices(wq_cache, ("q", 0), WqT_d, 0, 128, "wqs", 18)
                for sc in range(S // 512):
                    for t in range(NDT):
                        nc.sync.dma_start(
                            out=xT[t][:, sc * 512:(sc + 1) * 512],
                            in_=xT_d[t * 128:(t + 1) * 128,
                                     sc * 512:(sc + 1) * 512])
                with tc.psum_pool(name="bb", bufs=1) as bb:
                    # warmup: the PE clock needs ~3.4us of continuous work
                    # to leave the low pstate; spin on the identity tile
                    # while the startup DMAs are still in flight
                    warm = bb.tile([128, 128], F32, name="warm", tag="warm",
                                   bufs=1)
                    # ~100 spins cover the ~15us the startup DMAs (k0/q0
                    # weight slices + first x chunk) take to land
                    for _ in range(96):
                        nc.tensor.matmul(warm, ident, ident,
                                         start=True, stop=True)
                    for c in range(2):
                        bps = bb.tile([128, 512], F32, name="bps", tag="bb",
                                      bufs=2)
                        nc.tensor.matmul(bps, ones_c[0:1, :],
                                         bv_sb[0:1, c * 512:(c + 1) * 512],
                                         start=True, stop=True)
                        nc.vector.tensor_copy(
                            out=bv_bc[:, c * 512:(c + 1) * 512], in_=bps)
                with tc.psum_pool(name="pf", bufs=1) as pf:
                    psum_src.update(pool=pf, tag="pfg", bufs=2)
                    for sc in range(S // 512):
                        emit_k_group(0, sc)
                    emit_q_group(0, 0)
                # Wo is first consumed by out-proj around iteration 136;
                # loading it after the startup-critical x + k0/q0 slices
                # keeps the first projection groups off the DMA queue tail
                for t in range(NDT):
                    nc.sync.dma_start(out=Wo_t[t],
                                      in_=WoT_d[t * 128:(t + 1) * 128, :])
                _cms = [tc.psum_pool(name="sp", bufs=2),
                        tc.psum_pool(name="cp", bufs=2),
                        tc.psum_pool(name="op", bufs=1)]
                sp, cp, op = [cm.__enter__() for cm in _cms]
                psum_src.update(pool=op, tag="op", bufs=2)

                # filler order chosen so each tensor lands ahead of its
                # first consumer: vv[kt] heads 0,1 needed at iter kt;
                # kT[et]/qT2[et] at iter 16*et; vv heads 4c.. at iter 32c
                fillers = []
                fillers += [(emit_v_group, st, 0) for st in range(NST)]
                fillers += [(emit_k_group, 1, sc) for sc in range(4)]
                fillers.append((emit_q_group, 1, 0))
                fillers += [(emit_v_group, st, 1) for st in range(NST)]
                for et in (2, 3):
                    fillers += [(emit_k_group, et, sc) for sc in range(4)]
                    fillers.append((emit_q_group, et, 0))
                fillers += [(emit_v_group, st, 2) for st in range(NST)]
                for et in (4, 5):
                    fillers += [(emit_k_group, et, sc) for sc in range(4)]
                    fillers.append((emit_q_group, et, 0))
                fillers += [(emit_v_group, st, 3) for st in range(NST)]
                for et in (6, 7):
                    fillers += [(emit_k_group, et, sc) for sc in range(4)]
                    fillers.append((emit_q_group, et, 0))
                fillers = fillers[::-1]  # pop from the end
                # q projections for the second q-chunk aren't consumed until
                # qc1 (iteration 128+16*ht): drip them into the post-filler
                # bubble where ScalarE paces and the PE has slack
                late_fillers = [(emit_q_group, et, 1) for et in range(NET)]
                late_fillers = late_fillers[::-1]

                # ---------- attention ----------
                def emit_sc(qc, ht, kt):
                    # two K=64 matmuls on PE row groups 0/64 run
                    # concurrently (tile_position auto-derives from the
                    # operands' base partition): head 2ht -> cols 0:512,
                    # head 2ht+1 -> cols 512:1024 of the same psum tile
                    sc_ps = sp.tile([128, 1024], F32, name="sc_ps",
                                    tag="sc", bufs=2)
                    ksl = slice(kt * 128, (kt + 1) * 128)
                    qsl = slice(qc * 512, (qc + 1) * 512)
                    nc.tensor.matmul(
                        sc_ps[:, 0:512],
                        kT[ht][0:64, ksl], qT2[ht][0:64, qsl],
                        start=True, stop=True)
                    nc.tensor.matmul(
                        sc_ps[:, 512:1024],
                        kT[ht][64:128, ksl], qT2[ht][64:128, qsl],
                        start=True, stop=True)
                    return sc_ps

                def emit_outproj(qc_o, et):
                    ps = op.tile([128, 512], F32, name="ops", tag="op",
                                 bufs=2)
                    for dt in range(NDT):
                        nc.tensor.matmul(
                            ps,
                            Wo_t[dt][:, et * 128:(et + 1) * 128],
                            ctxn[qc_o][dt][:, :],
                            start=(dt == 0), stop=(dt == NDT - 1))
                    osb = p2.tile([128, 512], F32, name="osb", tag="osb",
                                  bufs=2)
                    nc.vector.tensor_copy(out=osb, in_=ps)
                    nc.gpsimd.dma_start(
                        out=outT_d[et * 128:(et + 1) * 128,
                                   qc_o * 512:(qc_o + 1) * 512],
                        in_=osb)

                def emit_outproj_partial(et):
                    # qc1 out-proj, dt 0..6 only; parked in SBUF as bf16 so
                    # the tail is just the dt=7 matmul + DVE add per e-tile
                    ps = op.tile([128, 512], F32, name="ops", tag="op",
                                 bufs=2)
                    for dt in range(NDT - 1):
                        nc.tensor.matmul(
                            ps,
                            Wo_t[dt][:, et * 128:(et + 1) * 128],
                            ctxn[1][dt][:, :],
                            start=(dt == 0), stop=(dt == NDT - 2))
                    nc.vector.tensor_copy(out=osb_part[et], in_=ps)

                def emit_outproj_final(et):
                    ps = op.tile([128, 512], F32, name="ops", tag="op",
                                 bufs=2)
                    nc.tensor.matmul(
                        ps,
                        Wo_t[NDT - 1][:, et * 128:(et + 1) * 128],
                        ctxn[1][NDT - 1][:, :],
                        start=True, stop=True)
                    osb = p2.tile([128, 512], F32, name="osb", tag="osb",
                                  bufs=2)
                    nc.vector.tensor_add(out=osb, in0=ps, in1=osb_part[et])
                    # tail DMA is the drain-critical path: split each
                    # 256KB store into 4 column chunks so they spread
                    # across 4 DMA queues (one queue moves ~22GB/s)
                    for cq in range(4):
                        nc.sync.dma_start(
                            out=outT_d[et * 128:(et + 1) * 128,
                                       512 + cq * 128:512 + (cq + 1) * 128],
                            in_=osb[:, cq * 128:(cq + 1) * 128])

                def emit_norm(ctq_sb, qc, h):
                    # per-q denominators sit per-PARTITION in flipped
                    # layout; normalize all 4 q-subtiles of one head
                    inv = p2.tile([128, 4, 1], F32, name="inv", tag="inv",
                                  bufs=3)
                    nc.vector.reciprocal_approx_fast(
                        inv, ctq_sb[:, :, HD:HD + 1])
                    ctqn = p2.tile([128, 4, HD], BF16, name="ctqn",
                                   tag="ctqn", bufs=4)
                    for qi in range(4):
                        nc.vector.tensor_scalar_mul(
                            out=ctqn[:, qi, :], in0=ctq_sb[:, qi, 0:HD],
                            scalar1=inv[:, qi, :])
                    tr_queue.append((ctqn, qc, h))

                def emit_tr(ctqn, qc, h):
                    # 4 back-to-back PE transposes (one per q-subtile) into
                    # a single psum tile borrowed from the out-proj slot
                    # rotation, then one [64,512] DVE copy into ctxn[d, q]
                    dtile, hp = h // 2, (h % 2) * 64
                    tp = op.tile([HD, 4, 128], BF16, name="tp", tag="op",
                                 bufs=2)
                    for qi in range(4):
                        nc.tensor.transpose(tp[:, qi, :], ctqn[:, qi, :],
                                            ident[:, :])
                    nc.vector.tensor_copy(
                        out=ctxn[qc][dtile][hp:hp + HD, :].rearrange(
                            "p (a c) -> p a c", a=4),
                        in_=tp)

                iters = [(qc, ht, kt)
                         for qc in range(NQC)
                         for ht in range(NHT)
                         for kt in range(NKT)]
                op_queue = []
                norm_queue = []
                tr_queue = []
                ctq_state = {}

                def emit_ctx_step(expT, qc, ht, kt):
                    # flipped ctx: expT slice stationary (M=128 q), v moving
                    # (N=65); 4 q-subtiles accumulate in one psum bank per
                    # head, both heads of the pair in flight
                    if kt == 0:
                        ctq_state["e"] = cp.tile([128, 4, HD + 1], F32,
                                                 name="ctqe", tag="ctq",
                                                 bufs=2)
                        ctq_state["o"] = cp.tile([128, 4, HD + 1], F32,
                                                 name="ctqo", tag="ctq",
                                                 bufs=2)
                    for half, key in ((0, "e"), (1, "o")):
                        ctq_ps = ctq_state[key]
                        h = 2 * ht + half
                        for qi in range(4):
                            # start only on the bank's first write and stop
                            # only on its last: the start/stop state covers
                            # the WHOLE 2KB bank, so sibling qi regions
                            # must not re-set or early-clear it
                            nc.tensor.matmul(
                                ctq_ps[:, qi, :],
                                expT[:, half * 512 + qi * 128:
                                     half * 512 + qi * 128 + 128],
                                vv[kt][:, h, :],
                                start=(kt == 0 and qi == 0),
                                stop=(kt == NKT - 1 and qi == 3))
                    if kt == NKT - 1:
                        for half, key in ((0, "e"), (1, "o")):
                            ctq_sb = p2.tile([128, 4, HD + 1], F32,
                                             name="ctq_sb", tag="ctq_sb",
                                             bufs=4)
                            nc.vector.tensor_copy(out=ctq_sb,
                                                  in_=ctq_state[key])
                            norm_queue.append((ctq_sb, qc, 2 * ht + half))
                        if qc == 0 and ht == NHT - 1:
                            op_queue.extend((0, et) for et in range(NET))

                sc_next = emit_sc(*iters[0])
                delayed = []
                partials = [et for et in range(NET)][::-1]
                for i, (qc, ht, kt) in enumerate(iters):
                    sc_ps = sc_next
                    expT = p2.tile([128, 1024], BF16, name="expT",
                                   tag="expT", bufs=6)
                    nc.scalar.activation(
                        expT, sc_ps,
                        mybir.ActivationFunctionType.Exp)
                    # ctx BEFORE the next scores pair: the 8 ctx LDWEIGHTS
                    # then prefetch during the previous iteration's filler
                    # matmuls instead of stalling behind the (shortened,
                    # row-tiled) scores window
                    delayed.append((expT, qc, ht, kt))
                    if len(delayed) > 1:
                        emit_ctx_step(*delayed.pop(0))
                    if i + 1 < len(iters):
                        sc_next = emit_sc(*iters[i + 1])
                    if norm_queue:
                        emit_norm(*norm_queue.pop(0))
                    if tr_queue:
                        emit_tr(*tr_queue.pop(0))
                    for _ in range(2 if i < 24 else 1):
                        if fillers:
                            fn, *args = fillers.pop()
                            fn(*args)
                    if i >= 96 and i % 16 == 0 and late_fillers:
                        fn, *args = late_fillers.pop()
                        fn(*args)
                    # qc0 out-proj: hidden in qc1's PE slack, 2 per block
                    if qc == 1 and kt in (8, 12) and op_queue:
                        emit_outproj(*op_queue.pop(0))
                    # qc1 out-proj partials during the last head-pair
                    # (kt>=4 so head-pair 6's norm/transpose chain has
                    # drained before the first partial's dt=6 matmul)
                    if qc == 1 and ht == NHT - 1 and kt >= 4 and partials:
                        emit_outproj_partial(partials.pop())
                while delayed:
                    emit_ctx_step(*delayed.pop(0))
                while norm_queue:
                    emit_norm(*norm_queue.pop(0))
                while tr_queue:
                    emit_tr(*tr_queue.pop(0))
                for args in op_queue:
                    emit_outproj(*args)
                while partials:
                    emit_outproj_partial(partials.pop())
                for et in range(NET):
                    emit_outproj_final(et)
                for cm in reversed(_cms):
                    cm.__exit__(None, None, None)
    nc.compile()
    return nc


def _get_nc():
    global _NC_CACHE
    if _NC_CACHE is None:
        _NC_CACHE = build_nc()
    return _NC_CACHE


def _prep_maps(x, Wq, bq, Wk, bk, Wv, bv, Wo):
    bf = ml_dtypes.bfloat16
    WqT = np.ascontiguousarray(Wq.T * SCALE).astype(bf)
    WkT = np.ascontiguousarray(Wk.T).astype(bf)
    WvT = np.ascontiguousarray(Wv.T).astype(bf)
    WoT = np.ascontiguousarray(Wo.T).astype(bf)
    bqt = np.ascontiguousarray(
        bq.reshape(NET, 128).T * SCALE).astype(np.float32)
    bkt = np.ascontiguousarray(bk.reshape(NET, 128).T).astype(np.float32)
    bvr = np.ascontiguousarray(bv.reshape(1, D)).astype(np.float32)
    in_maps = []
    for c in range(8):
        b, hq = c // 2, c % 2
        xTb = np.ascontiguousarray(x[b].T).astype(bf)  # [D, S]
        if hq == 1:
            # rotate so local query half sits at columns [0, SQ)
            xTb = np.ascontiguousarray(
                np.concatenate([xTb[:, SQ:], xTb[:, :SQ]], axis=1))
        in_maps.append(dict(xT=xTb, WqT=WqT, WkT=WkT, WvT=WvT, WoT=WoT,
                            bqt=bqt, bkt=bkt, bvr=bvr))
    return in_maps


def run(x, Wq, bq, Wk, bk, Wv, bv, Wo, bo, trace=False, **spmd_kwargs):
    nc = _get_nc()
    in_maps = _prep_maps(x, Wq, bq, Wk, bk, Wv, bv, Wo)
    res = run_bass_kernel_spmd(nc, in_maps, core_ids=list(range(8)),
                               trace=trace, **spmd_kwargs)
    out = np.empty((B, S, D), np.float32)
    for c in range(8):
        b, hq = c // 2, c % 2
        out[b, hq * SQ:(hq + 1) * SQ, :] = np.asarray(
            res.results[c]["outT"], np.float32).T
    out += bo.astype(np.float32)
    return out, res


def kernel(x, Wq, bq, Wk, bk, Wv, bv, Wo, bo):
    out, _ = run(np.asarray(x, np.float32), np.asarray(Wq, np.float32),
                 np.asarray(bq, np.float32), np.asarray(Wk, np.float32),
                 np.asarray(bk, np.float32), np.asarray(Wv, np.float32),
                 np.asarray(bv, np.float32), np.asarray(Wo, np.float32),
                 np.asarray(bo, np.float32))
    return out


# revision 5
# speedup vs baseline: 1.0551x; 1.0551x over previous
"""Multi-head attention (B=4, S=2048, D=1024, H=16) on 8 Trainium2 cores. v2

Sharding: core c -> (batch b=c//2, query-half hq=c%2). Each core computes
K/V projections for its batch's full sequence (no collectives needed) and
attention + output projection for its 1024 query rows.

v2 changes over the 432us baseline:
  * scores: two concurrent K=64 matmuls via PE row-tiling (tile_position
    (0,0)/(64,0) auto-derived from base partitions) instead of zero-padded
    K=128 contractions -- halves the scores PE time.  Iteration unit is
    (qc, head-pair ht, k-tile kt); the [128,1024] scores psum holds
    [head-even | head-odd] halves so the exp stays one [128,1024] ACT op.
  * q kept packed [128, SQ] per head-pair (qT2), no zero-padding memsets.
  * ctx normalization transposes moved from PE (identity matmuls through
    PSUM) to DVE 32x32 StreamTranspose ops -- frees ~35us of PE and one
    PSUM bank, letting scores/ctx/out-proj use all 8 banks.
  * x DMA'd chunk-major (s-chunks of 512 across all d-tiles) so the first
    K-projection group starts after 1MB instead of 4MB.
  * qc1 out-proj split: dt 0..6 partial-accumulated into SBUF during the
    last head's iterations, only the dt=7 matmul + DVE add in the tail.

Device dataflow (activations kept transposed, [feature, seq]):
  kT[e,s]   = WkT.T-contract  (lhsT=WkT[d,e] tiles, rhs=xT[d,s])
  qT2[ht]   = packed per-pair q [128, SQ]: head 2ht at partitions 0:64,
              head 2ht+1 at partitions 64:128.
  v[s,e]    = lhsT=xT[d,s] tiles, rhs=WvT[d,e]  (+bias via DVE add of a
              partition-broadcast bv); col 64 of each head = ones
              (softmax denominator column).
  per (qc, ht, kt):
    scoresT[k,q] for both heads of ht concurrently (row groups 0/64)
    expT = ScalarE Exp -> bf16 sbuf [128, 1024]
    ctx (flipped): ctq_h[q, hd+1] += expT-slice (stationary) @ vv[kt][:,h,:]
  norm: reciprocal_approx_fast [128,4] + per-qi tensor_scalar_mul -> ctqn
        bf16, then 8 DVE StreamTranspose ops scatter [q,d] -> ctxn[d,q].
  outT[e,q] = WoT.T-contract ctxn  (bias bo added host-side)
Host: out[b, hq*1024:(hq+1)*1024, :] = outT.T + bo
"""

import numpy as np
import ml_dtypes

import concourse.bacc as bacc
import concourse.tile as tile
from concourse import mybir
from concourse.bass_utils import run_bass_kernel_spmd
from concourse.masks import make_identity

B, S, D = 4, 2048, 1024
H, HD = 16, 64
SQ = 1024          # query rows per core
NDT = D // 128     # 8 d-tiles
NET = D // 128     # 8 e-tiles
NKT = S // 128     # 16 k-tiles
NST = S // 128     # 16 s-tiles
NQC = SQ // 512    # 2 q-chunks per core
NHT = H // 2       # 8 head-pairs
BF16 = mybir.dt.bfloat16
F32 = mybir.dt.float32
SCALE = 1.0 / 8.0  # 1/sqrt(HD)

_NC_CACHE = None


def build_nc():
    nc = bacc.Bacc(None, target_bir_lowering=False, debug=True)

    xT_d = nc.declare_dram_parameter("xT", [D, S], BF16, isOutput=False)
    WqT_d = nc.declare_dram_parameter("WqT", [D, D], BF16, isOutput=False)
    WkT_d = nc.declare_dram_parameter("WkT", [D, D], BF16, isOutput=False)
    WvT_d = nc.declare_dram_parameter("WvT", [D, D], BF16, isOutput=False)
    WoT_d = nc.declare_dram_parameter("WoT", [D, D], BF16, isOutput=False)
    bqt_d = nc.declare_dram_parameter("bqt", [128, NET], F32, isOutput=False)
    bkt_d = nc.declare_dram_parameter("bkt", [128, NET], F32, isOutput=False)
    bvr_d = nc.declare_dram_parameter("bvr", [1, D], F32, isOutput=False)
    outT_d = nc.declare_dram_parameter("outT", [D, SQ], BF16, isOutput=True)

    VC = 256           # v-projection chunk width (4 heads per chunk)
    NVC = D // VC      # 4 chunks

    with tile.TileContext(nc) as tc:
        with tc.tile_pool(name="resident", bufs=1) as res:
            # ---- resident SBUF tensors ----
            kT = [res.tile([128, S], BF16, name=f"kT{t}", tag=f"kT{t}")
                  for t in range(NET)]
            qT2 = [res.tile([128, SQ], BF16, name=f"qT2_{t}", tag=f"qT2_{t}")
                   for t in range(NHT)]
            vv = [res.tile([128, H, HD + 1], BF16, name=f"v{t}", tag=f"v{t}")
                  for t in range(NST)]
            ctxn = [[res.tile([128, 512], BF16, name=f"ctxn{qc}_{t}",
                              tag=f"ctxn{qc}_{t}") for t in range(NDT)]
                    for qc in range(NQC)]
            Wo_t = [res.tile([128, D], BF16, name=f"Wo{t}", tag=f"Wo{t}")
                    for t in range(NDT)]
            xT = [res.tile([128, S], BF16, name=f"xT{t}", tag=f"xT{t}")
                  for t in range(NDT)]
            osb_part = [res.tile([128, 512], BF16, name=f"osbp{t}",
                                 tag=f"osbp{t}") for t in range(NET)]
            bq_dma = res.tile([128, NET], F32, tag="bq_dma")
            bk_dma = res.tile([128, NET], F32, tag="bk_dma")
            bq_sb = res.tile([128, NET], F32, tag="bq_sb")
            bk_sb = res.tile([128, NET], F32, tag="bk_sb")
            bv_sb = res.tile([1, D], F32, tag="bv_sb")
            bv_bc = res.tile([128, D], F32, tag="bv_bc")
            ident = res.tile([128, 128], BF16, tag="ident")

            # identity first: it alone gates the PE warmup spins, which in
            # turn cover the several-us engine/DMA startup latency
            ones_c = res.tile([1, 128], F32, tag="ones_c")
            make_identity(nc, ident)
            nc.vector.memset(ones_c, 1.0)
            nc.sync.dma_start(out=bq_dma, in_=bqt_d[:, :])
            nc.sync.dma_start(out=bk_dma, in_=bkt_d[:, :])
            nc.sync.dma_start(out=bv_sb, in_=bvr_d[:, :])
            # TensorScalarPtr has a single sync-wait slot; route the biases
            # through DVE once so later readers rely on program order.
            nc.vector.tensor_copy(out=bq_sb, in_=bq_dma)
            nc.vector.tensor_copy(out=bk_sb, in_=bk_dma)
            for t in range(NST):
                # only the denominator column; cols 0:HD are overwritten
                nc.vector.memset(vv[t][:, :, HD:HD + 1], 1.0)

            with tc.tile_pool(name="p2", bufs=1) as p2:
                psum_src = {}

                def proj_ps():
                    return psum_src["pool"].tile(
                        [128, 512], F32, name="ps", tag=psum_src["tag"],
                        bufs=psum_src["bufs"])

                # ---------- projection emitters (also used as fillers) ----
                # weight SLICES are DMA-streamed per e-tile/chunk so that
                # three full weight sets never have to live in SBUF at once
                wk_cache = {}
                wq_cache = {}
                wv_cache = {}

                def w_slices(cache, key, W_d, c0, c1, tag, bufs):
                    if key not in cache:
                        ws = []
                        for dt in range(NDT):
                            wt = p2.tile([128, c1 - c0], BF16,
                                         name=f"{tag}{dt}", tag=tag,
                                         bufs=bufs)
                            nc.sync.dma_start(
                                out=wt, in_=W_d[dt * 128:(dt + 1) * 128,
                                                c0:c1])
                            ws.append(wt)
                        cache.clear()
                        cache[key] = ws
                    return cache[key]

                def emit_k_group(et, sc):
                    ws = w_slices(wk_cache, ("k", et), WkT_d,
                                  et * 128, (et + 1) * 128, "wks", 18)
                    ps = proj_ps()
                    for dt in range(NDT):
                        nc.tensor.matmul(
                            ps, ws[dt],
                            xT[dt][:, sc * 512: sc * 512 + 512],
                            start=(dt == 0), stop=(dt == NDT - 1))
                    nc.vector.tensor_scalar_add(
                        out=kT[et][:, sc * 512:(sc + 1) * 512],
                        in0=ps,
                        scalar1=bk_sb[:, et:et + 1])

                def emit_q_group(et, sc):
                    ws = w_slices(wq_cache, ("q", et), WqT_d,
                                  et * 128, (et + 1) * 128, "wqs", 18)
                    ps = proj_ps()
                    for dt in range(NDT):
                        nc.tensor.matmul(
                            ps, ws[dt],
                            xT[dt][:, sc * 512: sc * 512 + 512],
                            start=(dt == 0), stop=(dt == NDT - 1))
                    sl = slice(sc * 512, (sc + 1) * 512)
                    nc.vector.tensor_scalar_add(
                        out=qT2[et][0:64, sl],
                        in0=ps[0:64, :],
                        scalar1=bq_sb[0:64, et:et + 1])
                    nc.vector.tensor_scalar_add(
                        out=qT2[et][64:128, sl],
                        in0=ps[64:128, :],
                        scalar1=bq_sb[64:128, et:et + 1])

                def emit_v_group(st, c):
                    # v chunk c covers e-columns [c*VC, (c+1)*VC) = 4 heads
                    ws = w_slices(wv_cache, ("v", c), WvT_d,
                                  c * VC, (c + 1) * VC, "wvs", 18)
                    psw = proj_ps()
                    ps = psw[:, 0:VC]
                    for dt in range(NDT):
                        nc.tensor.matmul(
                            ps,
                            xT[dt][:, st * 128:(st + 1) * 128],
                            ws[dt],
                            start=(dt == 0), stop=(dt == NDT - 1))
                    nh = VC // HD
                    nc.vector.tensor_add(
                        out=vv[st][:, c * nh:(c + 1) * nh, 0:HD],
                        in0=ps.rearrange("p (h d) -> p h d", h=nh),
                        in1=bv_bc[:, c * VC:(c + 1) * VC].rearrange(
                            "p (h d) -> p h d", h=nh))

                # ---------- upfront: weights-for-k0/q0 and x chunk-major --
                # startup-critical DMA order: k0/q0 weight slices, then x
                # s-chunk-major so each k0 projection group only waits on
                # its chunk's 8 tiles (subtile deps)
                w_slices(wk_cache, ("k", 0), WkT_d, 0, 128, "wks", 18)
                w_slices(wq_cache, ("q", 0), WqT_d, 0, 128, "wqs", 18)
                for sc in range(S // 512):
                    for t in range(NDT):
                        nc.sync.dma_start(
                            out=xT[t][:, sc * 512:(sc + 1) * 512],
                            in_=xT_d[t * 128:(t + 1) * 128,
                                     sc * 512:(sc + 1) * 512])
                with tc.psum_pool(name="bb", bufs=1) as bb:
                    # warmup: the PE clock needs ~3.4us of continuous work
                    # to leave the low pstate; spin on the identity tile
                    # while the startup DMAs are still in flight
                    warm = bb.tile([128, 128], F32, name="warm", tag="warm",
                                   bufs=1)
                    # ~100 spins cover the ~15us the startup DMAs (k0/q0
                    # weight slices + first x chunk) take to land
                    for _ in range(144):
                        nc.tensor.matmul(warm, ident, ident,
                                         start=True, stop=True)
                    for c in range(2):
                        bps = bb.tile([128, 512], F32, name="bps", tag="bb",
                                      bufs=2)
                        nc.tensor.matmul(bps, ones_c[0:1, :],
                                         bv_sb[0:1, c * 512:(c + 1) * 512],
                                         start=True, stop=True)
                        nc.vector.tensor_copy(
                            out=bv_bc[:, c * 512:(c + 1) * 512], in_=bps)
                with tc.psum_pool(name="pf", bufs=1) as pf:
                    psum_src.update(pool=pf, tag="pfg", bufs=2)
                    for sc in range(S // 512):
                        emit_k_group(0, sc)
                    emit_q_group(0, 0)
                # Wo is first consumed by out-proj around iteration 136;
                # loading it after the startup-critical x + k0/q0 slices
                # keeps the first projection groups off the DMA queue tail
                for t in range(NDT):
                    nc.sync.dma_start(out=Wo_t[t],
                                      in_=WoT_d[t * 128:(t + 1) * 128, :])
                _cms = [tc.psum_pool(name="sp", bufs=2),
                        tc.psum_pool(name="cp", bufs=2),
                        tc.psum_pool(name="op", bufs=1)]
                sp, cp, op = [cm.__enter__() for cm in _cms]
                psum_src.update(pool=op, tag="op", bufs=2)

                # filler order chosen so each tensor lands ahead of its
                # first consumer: vv[kt] heads 0,1 needed at iter kt;
                # kT[et]/qT2[et] at iter 16*et; vv heads 4c.. at iter 32c
                fillers = []
                fillers += [(emit_v_group, st, 0) for st in range(NST)]
                fillers += [(emit_k_group, 1, sc) for sc in range(4)]
                fillers.append((emit_q_group, 1, 0))
                fillers += [(emit_v_group, st, 1) for st in range(NST)]
                for et in (2, 3):
                    fillers += [(emit_k_group, et, sc) for sc in range(4)]
                    fillers.append((emit_q_group, et, 0))
                fillers += [(emit_v_group, st, 2) for st in range(NST)]
                for et in (4, 5):
                    fillers += [(emit_k_group, et, sc) for sc in range(4)]
                    fillers.append((emit_q_group, et, 0))
                fillers += [(emit_v_group, st, 3) for st in range(NST)]
                for et in (6, 7):
                    fillers += [(emit_k_group, et, sc) for sc in range(4)]
                    fillers.append((emit_q_group, et, 0))
                fillers = fillers[::-1]  # pop from the end
                # q projections for the second q-chunk aren't consumed until
                # qc1 (iteration 128+16*ht): drip them into the post-filler
                # bubble where ScalarE paces and the PE has slack
                late_fillers = [(emit_q_group, et, 1) for et in range(NET)]
                late_fillers = late_fillers[::-1]

                # ---------- attention ----------
                def emit_sc(qc, ht, kt):
                    # two K=64 matmuls on PE row groups 0/64 run
                    # concurrently (tile_position auto-derives from the
                    # operands' base partition): head 2ht -> cols 0:512,
                    # head 2ht+1 -> cols 512:1024 of the same psum tile
                    sc_ps = sp.tile([128, 1024], F32, name="sc_ps",
                                    tag="sc", bufs=2)
                    ksl = slice(kt * 128, (kt + 1) * 128)
                    qsl = slice(qc * 512, (qc + 1) * 512)
                    nc.tensor.matmul(
                        sc_ps[:, 0:512],
                        kT[ht][0:64, ksl], qT2[ht][0:64, qsl],
                        start=True, stop=True)
                    nc.tensor.matmul(
                        sc_ps[:, 512:1024],
                        kT[ht][64:128, ksl], qT2[ht][64:128, qsl],
                        start=True, stop=True)
                    return sc_ps

                def emit_outproj(qc_o, et):
                    ps = op.tile([128, 512], F32, name="ops", tag="op",
                                 bufs=2)
                    for dt in range(NDT):
                        nc.tensor.matmul(
                            ps,
                            Wo_t[dt][:, et * 128:(et + 1) * 128],
                            ctxn[qc_o][dt][:, :],
                            start=(dt == 0), stop=(dt == NDT - 1))
                    osb = p2.tile([128, 512], BF16, name="osb", tag="osb",
                                  bufs=2)
                    nc.vector.tensor_copy(out=osb, in_=ps)
                    nc.gpsimd.dma_start(
                        out=outT_d[et * 128:(et + 1) * 128,
                                   qc_o * 512:(qc_o + 1) * 512],
                        in_=osb)

                def emit_outproj_partial(et):
                    # qc1 out-proj, dt 0..6 only; parked in SBUF as bf16 so
                    # the tail is just the dt=7 matmul + DVE add per e-tile
                    ps = op.tile([128, 512], F32, name="ops", tag="op",
                                 bufs=2)
                    for dt in range(NDT - 1):
                        nc.tensor.matmul(
                            ps,
                            Wo_t[dt][:, et * 128:(et + 1) * 128],
                            ctxn[1][dt][:, :],
                            start=(dt == 0), stop=(dt == NDT - 2))
                    nc.vector.tensor_copy(out=osb_part[et], in_=ps)

                def emit_outproj_final(et):
                    ps = op.tile([128, 512], F32, name="ops", tag="op",
                                 bufs=2)
                    nc.tensor.matmul(
                        ps,
                        Wo_t[NDT - 1][:, et * 128:(et + 1) * 128],
                        ctxn[1][NDT - 1][:, :],
                        start=True, stop=True)
                    osb = p2.tile([128, 512], BF16, name="osb", tag="osb",
                                  bufs=2)
                    nc.vector.tensor_add(out=osb, in0=ps, in1=osb_part[et])
                    nc.gpsimd.dma_start(
                        out=outT_d[et * 128:(et + 1) * 128, 512:1024],
                        in_=osb)

                def emit_norm(ctq_sb, qc, h):
                    # per-q denominators sit per-PARTITION in flipped
                    # layout; normalize all 4 q-subtiles of one head
                    inv = p2.tile([128, 4, 1], F32, name="inv", tag="inv",
                                  bufs=3)
                    nc.vector.reciprocal_approx_fast(
                        inv, ctq_sb[:, :, HD:HD + 1])
                    ctqn = p2.tile([128, 4, HD], BF16, name="ctqn",
                                   tag="ctqn", bufs=4)
                    for qi in range(4):
                        nc.vector.tensor_scalar_mul(
                            out=ctqn[:, qi, :], in0=ctq_sb[:, qi, 0:HD],
                            scalar1=inv[:, qi, :])
                    tr_queue.append((ctqn, qc, h))

                def emit_tr(ctqn, qc, h):
                    # 4 back-to-back PE transposes (one per q-subtile) into
                    # a single psum tile borrowed from the out-proj slot
                    # rotation, then one [64,512] DVE copy into ctxn[d, q]
                    dtile, hp = h // 2, (h % 2) * 64
                    tp = op.tile([HD, 4, 128], BF16, name="tp", tag="op",
                                 bufs=2)
                    for qi in range(4):
                        nc.tensor.transpose(tp[:, qi, :], ctqn[:, qi, :],
                                            ident[:, :])
                    nc.vector.tensor_copy(
                        out=ctxn[qc][dtile][hp:hp + HD, :].rearrange(
                            "p (a c) -> p a c", a=4),
                        in_=tp)

                iters = [(qc, ht, kt)
                         for qc in range(NQC)
                         for ht in range(NHT)
                         for kt in range(NKT)]
                op_queue = []
                norm_queue = []
                tr_queue = []
                ctq_state = {}

                def emit_ctx_step(expT, qc, ht, kt):
                    # flipped ctx: expT slice stationary (M=128 q), v moving
                    # (N=65); 4 q-subtiles accumulate in one psum bank per
                    # head, both heads of the pair in flight
                    if kt == 0:
                        ctq_state["e"] = cp.tile([128, 4, HD + 1], F32,
                                                 name="ctqe", tag="ctq",
                                                 bufs=2)
                        ctq_state["o"] = cp.tile([128, 4, HD + 1], F32,
                                                 name="ctqo", tag="ctq",
                                                 bufs=2)
                    for half, key in ((0, "e"), (1, "o")):
                        ctq_ps = ctq_state[key]
                        h = 2 * ht + half
                        for qi in range(4):
                            # start only on the bank's first write and stop
                            # only on its last: the start/stop state covers
                            # the WHOLE 2KB bank, so sibling qi regions
                            # must not re-set or early-clear it
                            nc.tensor.matmul(
                                ctq_ps[:, qi, :],
                                expT[:, half * 512 + qi * 128:
                                     half * 512 + qi * 128 + 128],
                                vv[kt][:, h, :],
                                start=(kt == 0 and qi == 0),
                                stop=(kt == NKT - 1 and qi == 3))
                    if kt == NKT - 1:
                        for half, key in ((0, "e"), (1, "o")):
                            ctq_sb = p2.tile([128, 4, HD + 1], F32,
                                             name="ctq_sb", tag="ctq_sb",
                                             bufs=4)
                            nc.vector.tensor_copy(out=ctq_sb,
                                                  in_=ctq_state[key])
                            norm_queue.append((ctq_sb, qc, 2 * ht + half))
                        if qc == 0 and ht == NHT - 1:
                            op_queue.extend((0, et) for et in range(NET))

                sc_next = emit_sc(*iters[0])
                delayed = []
                partials = [et for et in range(NET)][::-1]
                for i, (qc, ht, kt) in enumerate(iters):
                    sc_ps = sc_next
                    expT = p2.tile([128, 1024], BF16, name="expT",
                                   tag="expT", bufs=6)
                    nc.scalar.activation(
                        expT, sc_ps,
                        mybir.ActivationFunctionType.Exp)
                    # ctx BEFORE the next scores pair: the 8 ctx LDWEIGHTS
                    # then prefetch during the previous iteration's filler
                    # matmuls instead of stalling behind the (shortened,
                    # row-tiled) scores window.  Depth-2 delay: ctx(i-2)
                    # runs in iteration i so the PE never waits out the
                    # ~1.06us EXP latency of the tile it consumes
                    delayed.append((expT, qc, ht, kt))
                    if len(delayed) > 2:
                        emit_ctx_step(*delayed.pop(0))
                    if i + 1 < len(iters):
                        sc_next = emit_sc(*iters[i + 1])
                    if norm_queue:
                        emit_norm(*norm_queue.pop(0))
                    if tr_queue:
                        emit_tr(*tr_queue.pop(0))
                    for _ in range(2 if i < 24 else 1):
                        if fillers:
                            fn, *args = fillers.pop()
                            fn(*args)
                    if i >= 96 and i % 16 == 0 and late_fillers:
                        fn, *args = late_fillers.pop()
                        fn(*args)
                    # qc0 out-proj: hidden in qc1's PE slack, 2 per block
                    if qc == 1 and kt in (8, 12) and op_queue:
                        emit_outproj(*op_queue.pop(0))
                    # qc1 out-proj partials during the last head-pair
                    # (kt>=4 so head-pair 6's norm/transpose chain has
                    # drained before the first partial's dt=6 matmul)
                    if qc == 1 and ht == NHT - 1 and kt >= 4 and partials:
                        emit_outproj_partial(partials.pop())
                while delayed:
                    emit_ctx_step(*delayed.pop(0))
                while norm_queue:
                    emit_norm(*norm_queue.pop(0))
                while tr_queue:
                    emit_tr(*tr_queue.pop(0))
                for args in op_queue:
                    emit_outproj(*args)
                while partials:
                    emit_outproj_partial(partials.pop())
                for et in range(NET):
                    emit_outproj_final(et)
                for cm in reversed(_cms):
                    cm.__exit__(None, None, None)
    nc.compile()
    return nc


def _get_nc():
    global _NC_CACHE
    if _NC_CACHE is None:
        _NC_CACHE = build_nc()
    return _NC_CACHE


def _prep_maps(x, Wq, bq, Wk, bk, Wv, bv, Wo):
    bf = ml_dtypes.bfloat16
    WqT = np.ascontiguousarray(Wq.T * SCALE).astype(bf)
    WkT = np.ascontiguousarray(Wk.T).astype(bf)
    WvT = np.ascontiguousarray(Wv.T).astype(bf)
    WoT = np.ascontiguousarray(Wo.T).astype(bf)
    bqt = np.ascontiguousarray(
        bq.reshape(NET, 128).T * SCALE).astype(np.float32)
    bkt = np.ascontiguousarray(bk.reshape(NET, 128).T).astype(np.float32)
    bvr = np.ascontiguousarray(bv.reshape(1, D)).astype(np.float32)
    in_maps = []
    for c in range(8):
        b, hq = c // 2, c % 2
        xTb = np.ascontiguousarray(x[b].T).astype(bf)  # [D, S]
        if hq == 1:
            # rotate so local query half sits at columns [0, SQ)
            xTb = np.ascontiguousarray(
                np.concatenate([xTb[:, SQ:], xTb[:, :SQ]], axis=1))
        in_maps.append(dict(xT=xTb, WqT=WqT, WkT=WkT, WvT=WvT, WoT=WoT,
                            bqt=bqt, bkt=bkt, bvr=bvr))
    return in_maps


def run(x, Wq, bq, Wk, bk, Wv, bv, Wo, bo, trace=False, **spmd_kwargs):
    nc = _get_nc()
    in_maps = _prep_maps(x, Wq, bq, Wk, bk, Wv, bv, Wo)
    res = run_bass_kernel_spmd(nc, in_maps, core_ids=list(range(8)),
                               trace=trace, **spmd_kwargs)
    out = np.empty((B, S, D), np.float32)
    for c in range(8):
        b, hq = c // 2, c % 2
        out[b, hq * SQ:(hq + 1) * SQ, :] = np.asarray(
            res.results[c]["outT"], np.float32).T
    out += bo.astype(np.float32)
    return out, res


def kernel(x, Wq, bq, Wk, bk, Wv, bv, Wo, bo):
    out, _ = run(np.asarray(x, np.float32), np.asarray(Wq, np.float32),
                 np.asarray(bq, np.float32), np.asarray(Wk, np.float32),
                 np.asarray(bk, np.float32), np.asarray(Wv, np.float32),
                 np.asarray(bv, np.float32), np.asarray(Wo, np.float32),
                 np.asarray(bo, np.float32))
    return out


# revision 7
# speedup vs baseline: 1.0771x; 1.0209x over previous
"""Multi-head attention (B=4, S=2048, D=1024, H=16) on 8 Trainium2 cores.

Sharding: core c -> (batch b=c//2, query-half hq=c%2). Each core computes
K/V projections for its batch's full sequence (no collectives needed) and
attention + output projection for its 1024 query rows.

Device dataflow (activations kept transposed, [feature, seq], except ctx):
  kT[e,s]    = WkT.T-contract  (lhsT=WkT[d,e] tiles, rhs=xT[d,s])
  qTz[h]     = per-head zero-padded q [128, SQ]: head h's 64 dims at
               partitions (h%2)*64, rest zero.  Scores then contract over
               the full K=128 partitions (K=64 matmuls stream at half rate
               on trn2; zero rows make K=128 exact and full speed).
  v[s,e]     = lhsT=xT[d,s] tiles, rhs=WvT[d,e]  (+bias via DVE add of a
               partition-broadcast bv)
  per (q-chunk qc of 512, head h, k-pair kh):
    scoresT[k,q] = kT.T-contract qTz  (2 matmuls/kh -> [128,1024] psum)
    expT = ScalarE Exp(scale=0.125) -> bf16 sbuf
    flipped ctx: for each 128-q subtile qi: ctq[q,hd+1] += expT-slice
               (stationary, M=128) @ vv[kt][:,h,:] (moving, N=65);
               col 64 = softmax denominator (ones column of vv)
    norm: reciprocal_approx_fast [128,1] + tensor_scalar_mul (per-q denom
               is per-PARTITION in this layout), PE-transpose back to
               ctxn[d, q]
  outT[e,q]  = WoT.T-contract ctxn  (bias bo added host-side)
Projections for e-tiles >= 1 and v-chunks >= 1 are emitted as PE "filler"
groups inside the attention loop so the PE works while ScalarE exps pace
the attention pipeline.
Host: out[b, hq*1024:(hq+1)*1024, :] = outT.T + bo
"""

import numpy as np
import ml_dtypes

import concourse.bacc as bacc
import concourse.tile as tile
from concourse import mybir
from concourse.bass_utils import run_bass_kernel_spmd
from concourse.masks import make_identity

B, S, D = 4, 2048, 1024
H, HD = 16, 64
SQ = 1024          # query rows per core
NDT = D // 128     # 8 d-tiles
NET = D // 128     # 8 e-tiles
NKT = S // 128     # 16 k-tiles
NST = S // 128     # 16 s-tiles
NQC = SQ // 512    # 2 q-chunks per core
BF16 = mybir.dt.bfloat16
F32 = mybir.dt.float32
SCALE = 1.0 / 8.0  # 1/sqrt(HD)

_NC_CACHE = None


def build_nc():
    nc = bacc.Bacc(None, target_bir_lowering=False, debug=True)

    xT_d = nc.declare_dram_parameter("xT", [D, S], BF16, isOutput=False)
    WqT_d = nc.declare_dram_parameter("WqT", [D, D], BF16, isOutput=False)
    WkT_d = nc.declare_dram_parameter("WkT", [D, D], BF16, isOutput=False)
    WvT_d = nc.declare_dram_parameter("WvT", [D, D], BF16, isOutput=False)
    WoT_d = nc.declare_dram_parameter("WoT", [D, D], BF16, isOutput=False)
    bqt_d = nc.declare_dram_parameter("bqt", [128, NET], F32, isOutput=False)
    bkt_d = nc.declare_dram_parameter("bkt", [128, NET], F32, isOutput=False)
    bvr_d = nc.declare_dram_parameter("bvr", [1, D], F32, isOutput=False)
    outT_d = nc.declare_dram_parameter("outT", [D, SQ], BF16, isOutput=True)

    VC = 256           # v-projection chunk width (4 heads per chunk)
    NVC = D // VC      # 4 chunks

    with tile.TileContext(nc) as tc:
        with tc.tile_pool(name="resident", bufs=1) as res:
            # ---- resident SBUF tensors ----
            kT = [res.tile([128, S], BF16, name=f"kT{t}", tag=f"kT{t}")
                  for t in range(NET)]
            qTz = [res.tile([128, SQ], BF16, name=f"qTz{h}", tag=f"qTz{h}")
                   for h in range(H)]
            vv = [res.tile([128, H, HD + 1], BF16, name=f"v{t}", tag=f"v{t}")
                  for t in range(NST)]
            ctxn = [[res.tile([128, 512], BF16, name=f"ctxn{qc}_{t}",
                              tag=f"ctxn{qc}_{t}") for t in range(NDT)]
                    for qc in range(NQC)]
            Wo_t = [res.tile([128, D], BF16, name=f"Wo{t}", tag=f"Wo{t}")
                    for t in range(NDT)]
            xT = [res.tile([128, S], BF16, name=f"xT{t}", tag=f"xT{t}")
                  for t in range(NDT)]
            bq_dma = res.tile([128, NET], F32, tag="bq_dma")
            bk_dma = res.tile([128, NET], F32, tag="bk_dma")
            bq_sb = res.tile([128, NET], F32, tag="bq_sb")
            bk_sb = res.tile([128, NET], F32, tag="bk_sb")
            bv_sb = res.tile([1, D], F32, tag="bv_sb")
            bv_bc = res.tile([128, D], F32, tag="bv_bc")
            ident = res.tile([128, 128], BF16, tag="ident")

            # identity first: it alone gates the PE warmup spins, which
            # cover the several-us engine/DMA startup latency
            ones_c = res.tile([1, 128], F32, tag="ones_c")
            make_identity(nc, ident)
            nc.vector.memset(ones_c, 1.0)
            nc.sync.dma_start(out=bq_dma, in_=bqt_d[:, :])
            nc.sync.dma_start(out=bk_dma, in_=bkt_d[:, :])
            nc.sync.dma_start(out=bv_sb, in_=bvr_d[:, :])
            # TensorScalarPtr has a single sync-wait slot; route the biases
            # through DVE once so later readers rely on program order.
            nc.vector.tensor_copy(out=bq_sb, in_=bq_dma)
            nc.vector.tensor_copy(out=bk_sb, in_=bk_dma)
            for h in range(H):
                z0 = 64 if h % 2 == 0 else 0
                nc.vector.memset(qTz[h][z0:z0 + 64, :], 0.0)
            for t in range(NST):
                # only the denominator column; cols 0:HD are overwritten
                nc.vector.memset(vv[t][:, :, HD:HD + 1], 1.0)


            with tc.tile_pool(name="p2", bufs=1) as p2:
                with tc.psum_pool(name="bb", bufs=1) as bb:
                    for c in range(2):
                        bps = bb.tile([128, 512], F32, name="bps", tag="bb",
                                      bufs=2)
                        nc.tensor.matmul(bps, ones_c[0:1, :],
                                         bv_sb[0:1, c * 512:(c + 1) * 512],
                                         start=True, stop=True)
                        nc.vector.tensor_copy(
                            out=bv_bc[:, c * 512:(c + 1) * 512], in_=bps)
                psum_src = {}

                def proj_ps():
                    return psum_src["pool"].tile(
                        [128, 512], F32, name="ps", tag=psum_src["tag"],
                        bufs=psum_src["bufs"])

                # ---------- projection emitters (also used as fillers) ----
                # weight SLICES are DMA-streamed per e-tile/chunk so that
                # three full weight sets never have to live in SBUF at once
                wk_cache = {}
                wq_cache = {}
                wv_cache = {}

                def w_slices(cache, key, W_d, c0, c1, tag, bufs):
                    if key not in cache:
                        ws = []
                        for dt in range(NDT):
                            wt = p2.tile([128, c1 - c0], BF16,
                                         name=f"{tag}{dt}", tag=tag,
                                         bufs=bufs)
                            nc.sync.dma_start(
                                out=wt, in_=W_d[dt * 128:(dt + 1) * 128,
                                                c0:c1])
                            ws.append(wt)
                        cache.clear()
                        cache[key] = ws
                    return cache[key]

                def emit_k_group(et, sc):
                    ws = w_slices(wk_cache, ("k", et), WkT_d,
                                  et * 128, (et + 1) * 128, "wks", 18)
                    ps = proj_ps()
                    for dt in range(NDT):
                        nc.tensor.matmul(
                            ps, ws[dt],
                            xT[dt][:, sc * 512: sc * 512 + 512],
                            start=(dt == 0), stop=(dt == NDT - 1))
                    nc.vector.tensor_scalar_add(
                        out=kT[et][:, sc * 512:(sc + 1) * 512],
                        in0=ps,
                        scalar1=bk_sb[:, et:et + 1])

                def emit_q_group(et, sc):
                    ws = w_slices(wq_cache, ("q", et), WqT_d,
                                  et * 128, (et + 1) * 128, "wqs", 18)
                    ps = proj_ps()
                    for dt in range(NDT):
                        nc.tensor.matmul(
                            ps, ws[dt],
                            xT[dt][:, sc * 512: sc * 512 + 512],
                            start=(dt == 0), stop=(dt == NDT - 1))
                    sl = slice(sc * 512, (sc + 1) * 512)
                    nc.vector.tensor_scalar_add(
                        out=qTz[2 * et][0:64, sl],
                        in0=ps[0:64, :],
                        scalar1=bq_sb[0:64, et:et + 1])
                    nc.vector.tensor_scalar_add(
                        out=qTz[2 * et + 1][64:128, sl],
                        in0=ps[64:128, :],
                        scalar1=bq_sb[64:128, et:et + 1])

                def emit_v_group(st, c):
                    # v chunk c covers e-columns [c*VC, (c+1)*VC) = 4 heads
                    ws = w_slices(wv_cache, ("v", c), WvT_d,
                                  c * VC, (c + 1) * VC, "wvs", 18)
                    psw = proj_ps()
                    ps = psw[:, 0:VC]
                    for dt in range(NDT):
                        nc.tensor.matmul(
                            ps,
                            xT[dt][:, st * 128:(st + 1) * 128],
                            ws[dt],
                            start=(dt == 0), stop=(dt == NDT - 1))
                    nh = VC // HD
                    nc.vector.tensor_add(
                        out=vv[st][:, c * nh:(c + 1) * nh, 0:HD],
                        in0=ps.rearrange("p (h d) -> p h d", h=nh),
                        in1=bv_bc[:, c * VC:(c + 1) * VC].rearrange(
                            "p (h d) -> p h d", h=nh))

                # ---------- upfront: k0/q0 weights then x chunk-major --
                w_slices(wk_cache, ("k", 0), WkT_d, 0, 128, "wks", 18)
                w_slices(wq_cache, ("q", 0), WqT_d, 0, 128, "wqs", 18)
                for sc in range(S // 512):
                    for t in range(NDT):
                        nc.sync.dma_start(
                            out=xT[t][:, sc * 512:(sc + 1) * 512],
                            in_=xT_d[t * 128:(t + 1) * 128,
                                     sc * 512:(sc + 1) * 512])
                with tc.psum_pool(name="pf", bufs=1) as pf:
                    psum_src.update(pool=pf, tag="pfg", bufs=2)
                    # warmup: the PE clock needs ~3.4us of continuous work
                    # to leave the low pstate; ~144 spins also cover the
                    # ~20us the startup DMAs take to land so the first
                    # projection groups run warm
                    warm = pf.tile([128, 128], F32, name="warm", tag="warm",
                                   bufs=1)
                    for _ in range(144):
                        nc.tensor.matmul(warm, ident, ident,
                                         start=True, stop=True)
                    for sc in range(S // 512):
                        emit_k_group(0, sc)
                    emit_q_group(0, 0)
                # Wo is first consumed by out-proj around iteration 134;
                # loading it after the startup-critical x + k0/q0 slices
                # keeps the first projection groups off the DMA queue tail
                for t in range(NDT):
                    nc.sync.dma_start(out=Wo_t[t],
                                      in_=WoT_d[t * 128:(t + 1) * 128, :])
                _cms = [tc.psum_pool(name="sp", bufs=2),
                        tc.psum_pool(name="cp", bufs=2),
                        tc.psum_pool(name="op", bufs=1),
                        tc.psum_pool(name="tp", bufs=1)]
                sp, cp, op, tp = [cm.__enter__() for cm in _cms]
                psum_src.update(pool=op, tag="op", bufs=2)

                fillers = [(emit_v_group, st, 0) for st in range(NST)]
                for et in range(1, NET):
                    for sc in range(S // 512):
                        fillers.append((emit_k_group, et, sc))
                    fillers.append((emit_q_group, et, 0))
                    if et in (2, 4, 6):
                        c = et // 2
                        for st in range(NST):
                            fillers.append((emit_v_group, st, c))
                fillers = fillers[::-1]  # pop from the end
                # q projections for the second q-chunk aren't consumed until
                # qc1 (iteration 128+16*et): drip them into the post-filler
                # bubble where ScalarE paces and the PE has slack
                late_fillers = [(emit_q_group, et, 1) for et in range(NET)]
                late_fillers = late_fillers[::-1]

                # ---------- attention ----------
                def emit_sc(qc, h, kh):
                    ht = h // 2
                    sc_ps = sp.tile([128, 1024], F32, name="sc_ps",
                                    tag="sc", bufs=2)
                    for j in range(2):
                        kt = kh * 2 + j
                        nc.tensor.matmul(
                            sc_ps[:, j * 512:(j + 1) * 512],
                            kT[ht][:, kt * 128:(kt + 1) * 128],
                            qTz[h][:, qc * 512:(qc + 1) * 512],
                            start=True, stop=True)
                    return sc_ps

                def emit_outproj(qc_o, et):
                    ps = op.tile([128, 512], F32, name="ops", tag="op",
                                 bufs=2)
                    for dt in range(NDT):
                        nc.tensor.matmul(
                            ps,
                            Wo_t[dt][:, et * 128:(et + 1) * 128],
                            ctxn[qc_o][dt][:, :],
                            start=(dt == 0), stop=(dt == NDT - 1))
                    osb = p2.tile([128, 512], BF16, name="osb", tag="osb",
                                  bufs=2)
                    nc.vector.tensor_copy(out=osb, in_=ps)
                    nc.gpsimd.dma_start(
                        out=outT_d[et * 128:(et + 1) * 128,
                                   qc_o * 512:(qc_o + 1) * 512],
                        in_=osb)

                def emit_norm(ctq_ps, qi, qc, h):
                    # per-q denominators sit per-PARTITION in flipped layout
                    ht, hp = h // 2, (h % 2) * 64
                    inv = p2.tile([128, 1], F32, name="inv", tag="inv",
                                  bufs=3)
                    nc.vector.reciprocal_approx_fast(
                        inv, ctq_ps[:, qi, HD:HD + 1])
                    ctqn = p2.tile([128, HD], BF16, name="ctqn", tag="ctqn",
                                   bufs=3)
                    nc.vector.tensor_scalar_mul(
                        out=ctqn, in0=ctq_ps[:, qi, 0:HD], scalar1=inv)
                    tp_ps = tp.tile([HD, 128], BF16, name="tp_ps", tag="tp",
                                    bufs=1)
                    nc.tensor.transpose(tp_ps, ctqn, ident[:, :])
                    nc.vector.tensor_copy(
                        out=ctxn[qc][ht][hp:hp + HD, qi * 128:(qi + 1) * 128],
                        in_=tp_ps)

                iters = [(qc, h, kh)
                         for qc in range(NQC)
                         for h in range(H)
                         for kh in range(NKT // 2)]
                op_queue = []
                norm_queue = []
                ctq_state = {"ps": None}

                def emit_ctx_step(expT, qc, h, kh):
                    # flipped ctx: expT slice stationary (M=128 q), v moving
                    # (N=65); 4 q-subtiles accumulate in one psum bank
                    if kh == 0:
                        ctq_state["ps"] = cp.tile([128, 4, HD + 1], F32,
                                                  name="ctq", tag="ctq",
                                                  bufs=1)
                    ctq_ps = ctq_state["ps"]
                    for qi in range(4):
                        for j in range(2):
                            kt = kh * 2 + j
                            # start only on the bank's first write: the
                            # start bit marks the WHOLE 2KB bank pending-
                            # zero, so sibling qi regions must not re-set it
                            nc.tensor.matmul(
                                ctq_ps[:, qi, :],
                                expT[:, j * 512 + qi * 128:
                                     j * 512 + qi * 128 + 128],
                                vv[kt][:, h, :],
                                start=(kt == 0 and qi == 0),
                                stop=(kt == NKT - 1 and qi == 3))
                    if kh == NKT // 2 - 1:
                        ctq_sb = p2.tile([128, 4, HD + 1], F32,
                                         name="ctq_sb", tag="ctq_sb",
                                         bufs=2)
                        nc.vector.tensor_copy(out=ctq_sb, in_=ctq_ps)
                        for qi in range(4):
                            norm_queue.append((ctq_sb, qi, qc, h))
                        if qc == 0 and h == H - 1:
                            op_queue.extend((0, et) for et in range(NET))

                sc_next = emit_sc(*iters[0])
                delayed = []
                for i, (qc, h, kh) in enumerate(iters):
                    sc_ps = sc_next
                    expT = p2.tile([128, 1024], BF16, name="expT",
                                   tag="expT", bufs=6)
                    nc.scalar.activation(
                        expT, sc_ps,
                        mybir.ActivationFunctionType.Exp)
                    if i + 1 < len(iters):
                        sc_next = emit_sc(*iters[i + 1])
                    delayed.append((expT, qc, h, kh))
                    # depth-2: ctx(i-2) runs in iteration i so the PE does
                    # not wait out the ~1.06us EXP latency of its tile
                    if len(delayed) > 2:
                        emit_ctx_step(*delayed.pop(0))
                    if norm_queue:
                        emit_norm(*norm_queue.pop(0))
                    for _ in range(2 if i < 16 else 1):
                        if fillers:
                            fn, *args = fillers.pop()
                            fn(*args)
                    if i >= 96 and i % 18 == 0 and late_fillers:
                        fn, *args = late_fillers.pop()
                        fn(*args)
                    if kh == 6 and op_queue and h % 2 == 1:
                        emit_outproj(*op_queue.pop(0))
                while delayed:
                    emit_ctx_step(*delayed.pop(0))
                while norm_queue:
                    emit_norm(*norm_queue.pop(0))
                for args in op_queue:
                    emit_outproj(*args)
                for et in range(NET):
                    emit_outproj(1, et)
                for cm in reversed(_cms):
                    cm.__exit__(None, None, None)
    nc.compile()
    return nc


def _get_nc():
    global _NC_CACHE
    if _NC_CACHE is None:
        _NC_CACHE = build_nc()
    return _NC_CACHE


def _prep_maps(x, Wq, bq, Wk, bk, Wv, bv, Wo):
    bf = ml_dtypes.bfloat16
    WqT = np.ascontiguousarray(Wq.T * SCALE).astype(bf)
    WkT = np.ascontiguousarray(Wk.T).astype(bf)
    WvT = np.ascontiguousarray(Wv.T).astype(bf)
    WoT = np.ascontiguousarray(Wo.T).astype(bf)
    bqt = np.ascontiguousarray(
        bq.reshape(NET, 128).T * SCALE).astype(np.float32)
    bkt = np.ascontiguousarray(bk.reshape(NET, 128).T).astype(np.float32)
    bvr = np.ascontiguousarray(bv.reshape(1, D)).astype(np.float32)
    in_maps = []
    for c in range(8):
        b, hq = c // 2, c % 2
        xTb = np.ascontiguousarray(x[b].T).astype(bf)  # [D, S]
        if hq == 1:
            # rotate so local query half sits at columns [0, SQ)
            xTb = np.ascontiguousarray(
                np.concatenate([xTb[:, SQ:], xTb[:, :SQ]], axis=1))
        in_maps.append(dict(xT=xTb, WqT=WqT, WkT=WkT, WvT=WvT, WoT=WoT,
                            bqt=bqt, bkt=bkt, bvr=bvr))
    return in_maps


def run(x, Wq, bq, Wk, bk, Wv, bv, Wo, bo, trace=False, **spmd_kwargs):
    nc = _get_nc()
    in_maps = _prep_maps(x, Wq, bq, Wk, bk, Wv, bv, Wo)
    res = run_bass_kernel_spmd(nc, in_maps, core_ids=list(range(8)),
                               trace=trace, **spmd_kwargs)
    out = np.empty((B, S, D), np.float32)
    for c in range(8):
        b, hq = c // 2, c % 2
        out[b, hq * SQ:(hq + 1) * SQ, :] = np.asarray(
            res.results[c]["outT"], np.float32).T
    out += bo.astype(np.float32)
    return out, res


def kernel(x, Wq, bq, Wk, bk, Wv, bv, Wo, bo):
    out, _ = run(np.asarray(x, np.float32), np.asarray(Wq, np.float32),
                 np.asarray(bq, np.float32), np.asarray(Wk, np.float32),
                 np.asarray(bk, np.float32), np.asarray(Wv, np.float32),
                 np.asarray(bv, np.float32), np.asarray(Wo, np.float32),
                 np.asarray(bo, np.float32))
    return out

